# revision 19
# baseline (speedup 1.0000x reference)
"""Fused multi-head attention block (QKV proj + softmax attention + out-proj
+ LayerNorm) for Trainium2, sharded over 8 NeuronCores.

Sharding: tensor-parallel over heads. Core c owns heads [4c, 4c+4).
  - Each core computes q/k/v for its heads over the whole sequence
    (both batches), attention in S^T (keys-on-partitions) layout.
  - The kernel is scheduled as a continuous Scalar-engine exp stream
    (the hard bottleneck: 33.5M exps/core at 128 lanes x 1.2 GHz) with a
    software-pipelined S -> exp -> PV/denominator loop per key-tile, so
    the PE never sleeps long enough to lose its HAM warm clock.
  - Attention output is normalized (1/den) on the producer core, then an
    AllToAll (split into 4 chunks, overlapped with compute) reshards
    from head-parallel to row-parallel (4 x 128 rows per core); each core
    applies the 1024x1024 output projection + bias + LayerNorm.

dtypes: fp32 storage; QKV and S^T matmuls run as float32r (TF32-like,
full PE rate at N>=256); exp(S^T), PV and out-proj use bf16.
LayerNorm rstd uses exp(-0.5*ln(var+eps)) so the Scalar engine keeps a
single activation table set (natural_log_exp_and_others) loaded.
"""
import sys

for _p in ("/opt/trn_rl_repo", "/root/.axon_site/_ro/trn_rl_repo"):
    if _p not in sys.path:
        sys.path.insert(0, _p)

import numpy as np

import concourse.bass as bass
import concourse.tile as tile
from concourse import bacc, mybir
from concourse.masks import make_identity

F32 = mybir.dt.float32
F32R = mybir.dt.float32r
BF16 = mybir.dt.bfloat16
AF = mybir.ActivationFunctionType
ALU = mybir.AluOpType

N_CORES = 8
B, N, DIM = 2, 2048, 1024
HEADS, DH = 32, 32           # 32 heads x 32 dim/head
HPC = HEADS // N_CORES       # 4 heads per core
ROWS = B * N                 # 4096 global rows
SCALE = DH ** -0.5
EPS = 1e-6
KT = N // 128                # 16 key tiles per batch
QB = 512                     # q-block width
NQB = N // QB                # 4 q-blocks per batch
RC = 256                     # projection row-chunk
NRC = ROWS // RC             # 16 row chunks
NCK = 4                      # a2a chunks; chunk k = global rows [1024k, 1024k+1024)
                             # core c owns rows 1024k + 128c .. +128 of each chunk


def _build(debug=False):
    nc = bacc.Bacc("TRN2", target_bir_lowering=False, debug=False,
                   num_devices=N_CORES)

    xT_d = nc.dram_tensor("xT", [DIM, ROWS], F32R, kind="ExternalInput").ap()
    wqkv_d = nc.dram_tensor("wqkv", [DIM, 3 * HPC * DH], F32R,
                            kind="ExternalInput").ap()
    wout_d = nc.dram_tensor("wout", [DIM, DIM], F32, kind="ExternalInput").ap()
    bout_d = nc.dram_tensor("bout", [DIM], F32, kind="ExternalInput").ap()
    gamma_d = nc.dram_tensor("gamma", [DIM], F32, kind="ExternalInput").ap()
    beta_d = nc.dram_tensor("beta", [DIM], F32, kind="ExternalInput").ap()
    # rows: [chunk(4), 128]; global row = 1024*ck + 128*core + r
    out_d = nc.dram_tensor("out", [NCK * 128, DIM], F32,
                           kind="ExternalOutput").ap()
    if debug:
        dbg_qT = nc.dram_tensor("dbg_qT", [128, ROWS], F32,
                                kind="ExternalOutput").ap()
        dbg_kT = nc.dram_tensor("dbg_kT", [128, ROWS], F32,
                                kind="ExternalOutput").ap()
        dbg_V = nc.dram_tensor("dbg_V", [128, 2 * KT, 128], BF16,
                               kind="ExternalOutput").ap()
        dbg_att = nc.dram_tensor("dbg_att", [2 * NQB, 128, QB], BF16,
                                 kind="ExternalOutput").ap()
        dbg_rec = nc.dram_tensor("dbg_rec", [2 * NQB, 128, QB], F32,
                                 kind="ExternalOutput").ap()
        dbg_a2o = nc.dram_tensor("dbg_a2o", [NCK, N_CORES, 128, 128], BF16,
                                 kind="ExternalOutput").ap()

    with tile.TileContext(nc) as tc:
        with (
            tc.tile_pool(name="const", bufs=1) as const,
            tc.tile_pool(name="work", bufs=1) as work,
            tc.tile_pool(name="ps", bufs=1, space="PSUM") as ps,
            tc.tile_pool(name="dram", bufs=1, space="DRAM") as dram,
        ):
            # ---------------- constants / weights ----------------
            wqkv_sb = const.tile([128, 8, 3 * HPC * DH], F32R)
            nc.sync.dma_start(
                wqkv_sb[:], wqkv_d.rearrange("(kc p) m -> p kc m", p=128))
            ones_bf = const.tile([128, 1], BF16)
            nc.vector.memset(ones_bf[:], 1.0)
            ident = const.tile([128, 128], F32)
            make_identity(nc, ident[:])
            eps_sb = const.tile([128, 1], F32)
            nc.vector.memset(eps_sb[:], EPS)
            # head->partition-group broadcast matrix:
            # eb2[k, m] = 1 iff k == 32*(m//32); contracting against a
            # [128, q] tile whose rows 32h hold per-head denominators
            # broadcasts row 32h to output partitions [32h, 32h+32).
            eb2 = const.tile([128, 128], F32)
            nc.vector.memset(eb2[:], 0.0)
            for h in range(4):
                nc.vector.memset(eb2[32 * h:32 * h + 1, 32 * h:32 * h + 32],
                                 1.0)
            rec_full = const.tile([128, QB], F32)
            nc.vector.memset(rec_full[:], 0.0)
            dn_st = const.tile([128, QB], F32)
            nc.vector.memset(dn_st[:], 1.0)
            # warm the ACT table set (ln first so the shared
            # natural_log_exp_and_others set is chosen, then exp)
            scr = const.tile([128, 1], F32)
            nc.scalar.activation(out=scr[:], in_=eps_sb[:], func=AF.Ln,
                                 bias=eps_sb[:], scale=1.0)
            nc.scalar.activation(out=scr[:], in_=eps_sb[:], func=AF.Exp,
                                 scale=1.0)
            # row-broadcast vectors [128, 1024]
            bout_bc = const.tile([128, DIM], F32)
            nc.gpsimd.dma_start(out=bout_bc[:], in_=bass.AP(
                tensor=bout_d.tensor, offset=bout_d.offset,
                ap=[[0, 128], [1, DIM]]))
            gamma_bc = const.tile([128, DIM], F32)
            nc.gpsimd.dma_start(out=gamma_bc[:], in_=bass.AP(
                tensor=gamma_d.tensor, offset=gamma_d.offset,
                ap=[[0, 128], [1, DIM]]))
            beta_bc = const.tile([128, DIM], F32)
            nc.gpsimd.dma_start(out=beta_bc[:], in_=bass.AP(
                tensor=beta_d.tensor, offset=beta_d.offset,
                ap=[[0, 128], [1, DIM]]))
            # w_out -> bf16 [128, 8, 1024]
            wout_bf = const.tile([128, 8, DIM], BF16)

            # ---------------- persistent activations ----------------
            qT_sb = const.tile([128, ROWS], F32R)   # 4h x 32d on partitions
            kT_sb = const.tile([128, ROWS], F32R)
            V_sb = const.tile([128, 2 * KT, 128], BF16)  # [key%128, ktile, ch]

            # ---------------- dram bounce buffers ----------------
            a2a_in = [dram.tile([N_CORES, 128, 128], BF16, name=f"a2ai_{k}")
                      for k in range(NCK)]
            a2a_out = [dram.tile([N_CORES, 128, 128], BF16, name=f"a2ao_{k}")
                       for k in range(NCK)]

            # ---------------- phase A: projections ----------------
            def proj_rowchunk(rc):
                xt = work.tile([128, 8, RC], F32R, tag="xt", bufs=3,
                               name=f"xt_{rc}")
                dma_eng = nc.sync if rc % 2 == 0 else nc.gpsimd
                dma_eng.dma_start(
                    xt[:],
                    xT_d[:, rc * RC:(rc + 1) * RC]
                    .rearrange("(kc p) n -> p kc n", p=128))
                for name, mofs, dst in (("q", 0, qT_sb), ("k", 128, kT_sb)):
                    pp = ps.tile([128, RC], F32, tag="sp", bufs=3,
                                 name=f"pp_{name}_{rc}")
                    for kc in range(8):
                        nc.tensor.matmul(
                            pp[:], wqkv_sb[:, kc, mofs:mofs + 128],
                            xt[:, kc, :], start=(kc == 0), stop=(kc == 7))
                    nc.vector.tensor_copy(dst[:, rc * RC:(rc + 1) * RC], pp[:])
                # v: project (vT layout), cast bf16, DMA-transpose into V_sb
                pv_ = ps.tile([128, RC], F32, tag="sp", bufs=3,
                               name=f"pp_v_{rc}")
                for kc in range(8):
                    nc.tensor.matmul(
                        pv_[:], wqkv_sb[:, kc, 256:384], xt[:, kc, :],
                        start=(kc == 0), stop=(kc == 7))
                # one buffer per row-chunk: the async DMA-transpose read of
                # vt is not WAR-tracked, so never reuse these buffers
                vt = work.tile([128, RC], F32, tag="vt", bufs=2,
                               name=f"vt_{rc}")
                nc.vector.tensor_copy(vt[:], pv_[:])
                for i in range(RC // 128):
                    tp = ps.tile([128, 128], F32, tag="sp", bufs=3,
                                 name=f"tp_{rc}_{i}")
                    nc.tensor.matmul(
                        tp[:], vt[:, i * 128:(i + 1) * 128], ident[:],
                        is_transpose=True, start=True, stop=True)
                    nc.vector.tensor_copy(
                        V_sb[:, rc * (RC // 128) + i, :], tp[:])

            def load_wout(j):
                st = work.tile([128, DIM], F32, tag="wstage", bufs=2,
                               name=f"wst_{j}")
                nc.sync.dma_start(st[:], wout_d[j * 128:(j + 1) * 128, :])
                nc.vector.tensor_copy(wout_bf[:, j, :], st[:])

            for rc in range(2):             # just enough rows to start qb0
                proj_rowchunk(rc)

            # ---------------- phase B: attention ----------------
            # Software-pipelined per q-block: S two key-tiles ahead, exp
            # paces the loop, PV + denominator trail by one tile.
            def emit_S(b, qb, kt):
                q0 = b * N + qb * QB
                k0 = b * N + kt * 128
                tA = ps.tile([128, 2, QB], F32, tag="sp", bufs=3,
                             name=f"sA_{b}_{qb}_{kt}")
                tB = ps.tile([128, 2, QB], F32, tag="sp", bufs=3,
                             name=f"sB_{b}_{qb}_{kt}")
                for h in range(4):
                    t = tA if h < 2 else tB
                    nc.tensor.matmul(
                        t[:, h % 2, :],
                        kT_sb[32 * h:32 * h + 32, k0:k0 + 128],
                        qT_sb[32 * h:32 * h + 32, q0:q0 + QB],
                        start=True, stop=True, tile_position=(32 * h, 0))
                return tA, tB

            pending_epi = [None]

            def flush_epi():
                if pending_epi[0] is not None:
                    fn = pending_epi[0]
                    pending_epi[0] = None
                    fn()

            def attention_qblock(b, qb, extra=None):
                pvp = ps.tile([128, QB], F32, tag="pv", name=f"pv_{b}_{qb}")
                dnp = ps.tile([128, QB], F32, tag="dn", name=f"dn_{b}_{qb}")
                s_tiles = {0: emit_S(b, qb, 0), 1: emit_S(b, qb, 1)}
                flush_epi()   # previous q-block's tail, behind our first S
                for kt in range(KT):
                    if extra is not None:
                        extra(kt)
                    if kt + 2 < KT:
                        s_tiles[kt + 2] = emit_S(b, qb, kt + 2)
                    tA, tB = s_tiles.pop(kt)
                    eA = work.tile([128, 2, QB], BF16, tag="expt", bufs=6,
                                   name=f"eA_{b}_{qb}_{kt}")
                    eB = work.tile([128, 2, QB], BF16, tag="expt", bufs=6,
                                   name=f"eB_{b}_{qb}_{kt}")
                    nc.scalar.activation(eA[:], tA[:], AF.Exp, scale=SCALE)
                    nc.scalar.activation(eB[:], tB[:], AF.Exp, scale=SCALE)
                    for h in range(4):
                        rhs = (eA if h < 2 else eB)[:, h % 2, :]
                        nc.tensor.matmul(
                            pvp[32 * h:32 * h + 32, :],
                            V_sb[:, b * KT + kt, 32 * h:32 * h + 32],
                            rhs, start=(kt == 0), stop=(kt == KT - 1),
                            tile_position=(0, 32 * h))
                        nc.tensor.matmul(
                            dnp[32 * h:32 * h + 1, :],
                            ones_bf[:], rhs,
                            start=(kt == 0), stop=(kt == KT - 1),
                            tile_position=(0, 32 * h))
                # epilogue part 1: denominators off PSUM, reciprocal
                for h in range(4):
                    nc.vector.tensor_copy(dn_st[32 * h:32 * h + 1, :],
                                          dnp[32 * h:32 * h + 1, :])
                nc.vector.reciprocal_approx_fast(out=rec_full[:],
                                                 in_=dn_st[:])

                def epilogue():
                    recb = ps.tile([128, QB], F32, tag="sp", bufs=3,
                                   name=f"recb_{b}_{qb}")
                    nc.tensor.matmul(recb[:], eb2[:], rec_full[:],
                                     start=True, stop=True)
                    attf = work.tile([128, QB], F32, tag="attf", bufs=2,
                                     name=f"attf_{b}_{qb}")
                    nc.vector.tensor_copy(attf[:], pvp[:])
                    att = work.tile([128, QB], BF16, tag="att", bufs=2,
                                    name=f"att_{b}_{qb}")
                    nc.vector.tensor_tensor(att[:], attf[:], recb[:],
                                            ALU.mult)
                    ck = 2 * b + qb // 2
                    j0 = 4 * (qb % 2)
                    for j4 in range(4):
                        nc.sync.dma_start(a2a_in[ck][j0 + j4],
                                          att[:, 128 * j4:128 * j4 + 128])
                    if debug:
                        nc.sync.dma_start(dbg_att[b * NQB + qb], att[:])
                        nc.sync.dma_start(dbg_rec[b * NQB + qb], rec_full[:])

                pending_epi[0] = epilogue

            def a2a_exchange(ck):
                nc.gpsimd.collective_compute(
                    "AllToAll", ALU.bypass,
                    replica_groups=[list(range(N_CORES))],
                    ins=[a2a_in[ck].opt()], outs=[a2a_out[ck].opt()])

            # ---------------- phase C: out-proj + LN (per 128 rows) -------
            def outproj_chunk(ck):
                ab = work.tile([128, 8, 128], BF16, tag="a2asb", bufs=2,
                               name=f"ab_{ck}")
                for i in range(N_CORES):
                    nc.sync.dma_start(ab[:, i, :], a2a_out[ck][i])
                osb = work.tile([128, DIM], F32, tag="osb", bufs=2,
                                name=f"osb_{ck}")
                for nb in range(2):
                    op = ps.tile([128, 512], F32, tag="sp", bufs=3,
                                 name=f"op_{ck}_{nb}")
                    for i in range(N_CORES):
                        nc.tensor.matmul(
                            op[:], ab[:, i, :],
                            wout_bf[:, i, nb * 512:(nb + 1) * 512],
                            start=(i == 0), stop=(i == N_CORES - 1))
                    nc.vector.tensor_tensor(
                        osb[:, nb * 512:(nb + 1) * 512], op[:],
                        bout_bc[:, nb * 512:(nb + 1) * 512], ALU.add)
                # LayerNorm over the 1024 free dim
                stats = work.tile([128, 2, 6], F32, tag="stats", bufs=2,
                                  name=f"stats_{ck}")
                for sg in range(2):
                    nc.vector.bn_stats(out=stats[:, sg, :],
                                       in_=osb[:, sg * 512:(sg + 1) * 512])
                mv = work.tile([128, 2], F32, tag="mv", bufs=2,
                               name=f"mv_{ck}")
                nc.vector.bn_aggr(out=mv[:], in_=stats[:])
                # rstd = exp(-0.5 * ln(var + eps)) — stays in the exp/ln set
                lnv = work.tile([128, 1], F32, tag="lnv", bufs=2,
                                name=f"lnv_{ck}")
                nc.scalar.activation(out=lnv[:], in_=mv[:, 1:2], func=AF.Ln,
                                     bias=eps_sb[:], scale=1.0)
                rstd = work.tile([128, 1], F32, tag="rstd", bufs=2,
                                 name=f"rstd_{ck}")
                nc.scalar.activation(out=rstd[:], in_=lnv[:], func=AF.Exp,
                                     scale=-0.5)
                nc.vector.tensor_scalar(
                    out=osb[:], in0=osb[:], scalar1=mv[:, 0:1],
                    scalar2=rstd[:], op0=ALU.subtract, op1=ALU.mult)
                nc.vector.tensor_tensor(osb[:], osb[:], gamma_bc[:], ALU.mult)
                nc.vector.tensor_tensor(osb[:], osb[:], beta_bc[:], ALU.add)
                nc.sync.dma_start(out_d[ck * 128:(ck + 1) * 128, :], osb[:])

            # ---------------- schedule ----------------
            # per-kt emission hooks: stream the remaining projections and
            # weight loads into the attention pipeline instead of bursts
            def extra_00(kt):     # proj chunks 2..7 (rest of batch 0)
                if kt % 2 == 0 and kt // 2 + 2 < 8:
                    proj_rowchunk(kt // 2 + 2)

            def extra_01(kt):     # proj chunks 8..11
                if kt % 4 == 0:
                    proj_rowchunk(8 + kt // 4)

            def extra_02(kt):     # proj chunks 12..15
                if kt % 4 == 0:
                    proj_rowchunk(12 + kt // 4)

            def extra_03(kt):     # w_out rows 0..5
                if kt % 2 == 1 and kt // 2 < 6:
                    load_wout(kt // 2)

            def extra_10(kt):     # w_out rows 6,7
                if kt == 1 or kt == 3:
                    load_wout(6 + kt // 2)

            attention_qblock(0, 0, extra_00)
            attention_qblock(0, 1, extra_01)
            flush_epi()
            a2a_exchange(0)
            attention_qblock(0, 2, extra_02)
            attention_qblock(0, 3, extra_03)
            flush_epi()
            a2a_exchange(1)
            attention_qblock(1, 0, extra_10)
            attention_qblock(1, 1)
            flush_epi()
            a2a_exchange(2)
            outproj_chunk(0)
            attention_qblock(1, 2)
            outproj_chunk(1)
            attention_qblock(1, 3)
            flush_epi()
            a2a_exchange(3)
            outproj_chunk(2)
            outproj_chunk(3)
            if debug:
                nc.sync.dma_start(dbg_qT, qT_sb[:].bitcast(F32))
                nc.sync.dma_start(dbg_kT, kT_sb[:].bitcast(F32))
                nc.sync.dma_start(dbg_V, V_sb[:])
                for k in range(NCK):
                    sbk = work.tile([128, 8, 128], BF16, tag="a2asb", bufs=2,
                                    name=f"dbga_{k}")
                    for i in range(N_CORES):
                        nc.sync.dma_start(sbk[:, i, :], a2a_out[k][i])
                    nc.sync.dma_start(
                        dbg_a2o[k].rearrange("c p n -> p c n"), sbk[:])

    nc.compile()
    return nc


class _Runner:
    """Compile once; run the SPMD kernel on 8 cores via PJRT repeatedly."""

    def __init__(self):
        self.nc = _build()
        import jax
        from jax.sharding import Mesh, PartitionSpec, NamedSharding
        from jax.experimental.shard_map import shard_map
        from concourse import bass2jax
        bass2jax.install_neuronx_cc_hook()

        nc = self.nc
        part_name = (nc.partition_id_tensor.name
                     if nc.partition_id_tensor else None)
        in_names, out_names, out_avals = [], [], []
        for alloc in nc.m.functions[0].allocations:
            if not isinstance(alloc, mybir.MemoryLocationSet):
                continue
            name = alloc.memorylocations[0].name
            if alloc.kind == "ExternalInput":
                if name != part_name:
                    in_names.append(name)
            elif alloc.kind == "ExternalOutput":
                out_names.append(name)
                out_avals.append(jax.core.ShapedArray(
                    tuple(alloc.tensor_shape), mybir.dt.np(alloc.dtype)))
        self.in_names = list(in_names)
        self.out_names = out_names
        self.out_avals = out_avals
        all_in_names = in_names + out_names
        if part_name is not None:
            all_in_names = all_in_names + [part_name]

        def _body(*args):
            operands = list(args)
            if part_name is not None:
                operands.append(bass2jax.partition_id_tensor())
            outs = bass2jax._bass_exec_p.bind(
                *operands, out_avals=tuple(out_avals),
                in_names=tuple(all_in_names), out_names=tuple(out_names),
                lowering_input_output_aliases=(),
                sim_require_finite=True, sim_require_nnan=True, nc=nc)
            return tuple(outs)

        devices = jax.devices()[:N_CORES]
        mesh = Mesh(np.asarray(devices), ("core",))
        self.sharding = NamedSharding(mesh, PartitionSpec("core"))
        nin = len(self.in_names) + len(out_names)
        self.fn = jax.jit(shard_map(
            _body, mesh=mesh, in_specs=(PartitionSpec("core"),) * nin,
            out_specs=(PartitionSpec("core"),) * len(out_names),
            check_rep=False))
        self.jax = jax

    def stage(self, in_maps):
        """Concatenate per-core inputs + zero outputs; device_put with the
        mesh sharding so steady-state calls skip any resharding."""
        concat = [np.concatenate([m[name] for m in in_maps], axis=0)
                  for name in self.in_names]
        zeros = [np.zeros((N_CORES * a.shape[0], *a.shape[1:]), a.dtype)
                 for a in self.out_avals]
        return [self.jax.device_put(x, self.sharding) for x in concat + zeros]

    def run_staged(self, staged):
        outs = self.fn(*staged)
        self.jax.block_until_ready(outs)
        return outs

    def run(self, in_maps):
        outs = self.run_staged(self.stage(in_maps))
        return [
            {name: np.asarray(outs[i]).reshape(
                N_CORES, *self.out_avals[i].shape)[c]
             for i, name in enumerate(self.out_names)}
            for c in range(N_CORES)
        ]


_RUNNER = None


def _get_runner():
    global _RUNNER
    if _RUNNER is None:
        _RUNNER = _Runner()
    return _RUNNER


def _make_in_maps(x, w_qkv, w_out, b_out, ln_gamma, ln_beta):
    x = np.asarray(x, dtype=np.float32)
    w_qkv = np.asarray(w_qkv, dtype=np.float32)
    w_out = np.asarray(w_out, dtype=np.float32)
    b_out = np.asarray(b_out, dtype=np.float32)
    ln_gamma = np.asarray(ln_gamma, dtype=np.float32)
    ln_beta = np.asarray(ln_beta, dtype=np.float32)

    xT = np.ascontiguousarray(x.reshape(ROWS, DIM).T)
    in_maps = []
    for c in range(N_CORES):
        h0 = HPC * c * DH
        cols = np.concatenate([
            w_qkv[:, h0:h0 + HPC * DH],
            w_qkv[:, DIM + h0:DIM + h0 + HPC * DH],
            w_qkv[:, 2 * DIM + h0:2 * DIM + h0 + HPC * DH],
        ], axis=1)
        in_maps.append({
            "xT": xT,
            "wqkv": np.ascontiguousarray(cols),
            "wout": w_out,
            "bout": b_out,
            "gamma": ln_gamma,
            "beta": ln_beta,
        })
    return in_maps


def kernel(x, w_qkv, w_out, b_out, ln_gamma, ln_beta):
    runner = _get_runner()
    in_maps = _make_in_maps(x, w_qkv, w_out, b_out, ln_gamma, ln_beta)
    results = runner.run(in_maps)
    # per-core out rows: [chunk(4), 128]; global row = 1024*ck + 128*c + r
    full = np.empty((ROWS, DIM), dtype=np.float32)
    for c in range(N_CORES):
        o = results[c]["out"]
        for ck in range(NCK):
            r0 = 1024 * ck + 128 * c
            full[r0:r0 + 128] = o[ck * 128:(ck + 1) * 128]
    return full.reshape(B, N, DIM)
